# revision 14
# baseline (speedup 1.0000x reference)
"""Trainium2 Bass kernel for ContentAttention (sparse_attention).

reference semantics:
    logits = einsum("bqd,bkd->bqk", queries, keys)            # B,Q,K
    key_mask = (keys[:, :, 0] == 0.0)[:, None, :]             # mask keys whose feat0 == 0
    logits = where(key_mask, -inf, logits)
    lse = logsumexp(logits, -1)                               # B,Q
    confidence = sigmoid((lse + bias) * temperature)
    attn = softmax(logits, -1)
    returns (attn, confidence)

Strategy: data-parallel over batch; B=32 -> 4 batches per core on 8 cores.
Per batch on-device:
  - DMA Q[b], K[b] (512x1024) into SBUF in natural [q|k, d] layout
  - PE-transpose both into [d, q|k] layout (PSUM), evacuate to SBUF
    (ScalarE for Q^T, VectorE for K^T)
  - 4 output tiles [128q, 512k], each accumulating 8 d-chunk matmuls in PSUM
    + one rank-1 "mask bias" matmul (lhsT=ones[1,128], rhs=maskbias[1,512])
    which adds -1e30 to every masked key column (no-op when mask is empty)
  - softmax: DVE reduce_max(negate) -> ACT exp(x + negmax) with accum_out
    row-sum -> DVE reciprocal + tensor_scalar multiply
  - row (-max) and row sum are written to a stats tile; host finishes
    logsumexp + sigmoid for the confidence output (16K elements).

The whole Q/K path (DRAM tensors, SBUF tiles, identity, mask) is declared in
the matmul dtype: float32r (tf32-like, 1 PE cycle/row vs 4 for float32) by
default. The BIR verifier requires fp32r-matmul operands to be *produced* as
fp32r, so the dtype is set at allocation rather than bitcast at use.
"""

import os
import sys

if "/opt/trn_rl_repo" not in sys.path:
    sys.path.insert(0, "/opt/trn_rl_repo")

import numpy as np

import concourse.bass as bass
import concourse.mybir as mybir
import concourse.tile as tile
from concourse import bacc, bass_utils

B, Q, K, D = 32, 512, 512, 1024
N_CORES = 8
B_LOC = B // N_CORES  # batches per core
P = 128  # partitions
QT = Q // P  # q tiles per batch
DC = D // P  # d chunks

# matmul input dtype: float32r runs the PE at 1 cycle/row (vs 4 for float32)
# at ~tf32 precision. Overridable for experiments.
MM_DT_NAME = os.environ.get("ATTN_MM_DT", "f32r")
MM_DT = {
    "f32": mybir.dt.float32,
    "f32r": mybir.dt.float32r,
    "bf16": mybir.dt.bfloat16,
    "f16": mybir.dt.float16,
}[MM_DT_NAME]

MASK_NEG = -1.0e30


def build_kernel(b_loc: int = B_LOC, mm_dt=MM_DT):
    nc = bacc.Bacc("TRN2", target_bir_lowering=False, debug=False)
    f32 = mybir.dt.float32

    q_dram = nc.dram_tensor("queries", (b_loc, Q, D), mm_dt, kind="ExternalInput")
    k_dram = nc.dram_tensor("keys", (b_loc, K, D), mm_dt, kind="ExternalInput")
    mb_dram = nc.dram_tensor("maskbias", (b_loc, K), mm_dt, kind="ExternalInput")
    # identity (for PE transpose) and a ones row (for the rank-1 mask matmul)
    # come in via DRAM: memset/affine_select can't produce f32r-typed tiles
    id_dram = nc.dram_tensor("identity", (P, P), mm_dt, kind="ExternalInput")
    ones_dram = nc.dram_tensor("onesrow", (1, P), mm_dt, kind="ExternalInput")
    attn_dram = nc.dram_tensor("attn", (b_loc, Q, K), f32, kind="ExternalOutput")
    # stats[p, b*2*QT + 2*i + 0] = -rowmax(b, q=128*i+p)
    # stats[p, b*2*QT + 2*i + 1] = rowsum(b, q=128*i+p)
    stats_dram = nc.dram_tensor("stats", (P, b_loc * 2 * QT), f32, kind="ExternalOutput")

    with tile.TileContext(nc) as tc:
        with (
            tc.tile_pool(name="consts", bufs=1) as consts,
            tc.tile_pool(name="nat", bufs=3) as nat_pool,
            tc.tile_pool(name="tsb", bufs=DC) as tsb_pool,
            tc.tile_pool(name="attn", bufs=2) as attn_pool,
            tc.tile_pool(name="small", bufs=4) as small_pool,
            tc.tile_pool(name="stats", bufs=1) as stats_pool,
            tc.tile_pool(name="tps", bufs=2, space="PSUM") as tps_pool,
            tc.tile_pool(name="lg", bufs=4, space="PSUM") as lg_pool,
        ):
            identity = consts.tile([P, P], mm_dt)
            nc.sync.dma_start(identity[:], id_dram[:])
            ones_row = consts.tile([1, P], mm_dt)
            nc.sync.dma_start(ones_row[:], ones_dram[:])
            mb_sb = consts.tile([1, b_loc * K], mm_dt)
            nc.sync.dma_start(
                mb_sb[:], mb_dram[:].rearrange("b k -> (b k)").unsqueeze(0)
            )
            stats_sb = stats_pool.tile([P, b_loc * 2 * QT], f32)

            for b in range(b_loc):
                # ---- load Q[b], K[b] in natural layout [p, i, d] (q = i*128+p)
                # batch 0 loads in per-i-block pieces so the first transposes
                # start ~6us earlier; later batches load while compute runs
                n_pieces = QT if b == 0 else 1
                q_nat = nat_pool.tile([P, QT, D], mm_dt, tag="qnat")
                k_nat = nat_pool.tile([P, QT, D], mm_dt, tag="knat")
                for nat, dram in ((q_nat, q_dram), (k_nat, k_dram)):
                    src = dram[b].rearrange("(i p) d -> p i d", p=P)
                    step = QT // n_pieces
                    for piece in range(n_pieces):
                        s = slice(piece * step, (piece + 1) * step)
                        nc.sync.dma_start(nat[:, s, :], src[:, s, :])

                # ---- transpose to [d, q] / [d, k]; two d-chunks per psum tile
                qt_sb = []
                kt_sb = []
                for nat, out_list, tag, copy_eng in (
                    (q_nat, qt_sb, "qt", nc.scalar),
                    (k_nat, kt_sb, "kt", nc.vector),
                ):
                    for pair in range(DC // 2):
                        tps = tps_pool.tile([P, 2 * Q], mm_dt, tag="tps")
                        for half in range(2):
                            c = 2 * pair + half
                            for i in range(QT):
                                nc.tensor.transpose(
                                    tps[:, Q * half + P * i : Q * half + P * (i + 1)],
                                    nat[:, i, P * c : P * (c + 1)],
                                    identity[:],
                                )
                        t_sb = tsb_pool.tile([P, 2 * Q], mm_dt, tag=tag)
                        if copy_eng is nc.scalar:
                            nc.scalar.copy(t_sb[:], tps[:])
                        else:
                            nc.vector.tensor_copy(t_sb[:], tps[:])
                        out_list.append(t_sb)

                # ---- matmuls + softmax per q tile
                attn_sb = attn_pool.tile([P, QT, K], f32, tag="attn")
                for i in range(QT):
                    lg = lg_pool.tile([P, K], f32, tag="lg")
                    for c in range(DC):
                        pair, half = divmod(c, 2)
                        nc.tensor.matmul(
                            lg[:],
                            qt_sb[pair][:, Q * half + P * i : Q * half + P * (i + 1)],
                            kt_sb[pair][:, Q * half : Q * (half + 1)],
                            start=(c == 0),
                            stop=False,
                        )
                    # mask bias: logits[q, k] += 1 * maskbias[k]
                    nc.tensor.matmul(
                        lg[:],
                        ones_row[0:1, :],
                        mb_sb[0:1, K * b : K * (b + 1)],
                        start=False,
                        stop=True,
                    )

                    col = b * 2 * QT + 2 * i
                    negmax = stats_sb[:, col : col + 1]
                    rowsum = stats_sb[:, col + 1 : col + 2]
                    nc.vector.reduce_max(
                        negmax, lg[:], axis=mybir.AxisListType.X, negate=True
                    )
                    attn_i = attn_sb[:, i, :]
                    nc.scalar.activation(
                        attn_i,
                        lg[:],
                        mybir.ActivationFunctionType.Exp,
                        bias=negmax,
                        scale=1.0,
                        accum_out=rowsum,
                    )
                    recip = small_pool.tile([P, 1], f32, tag="recip")
                    nc.vector.reciprocal(recip[:], rowsum)
                    nc.vector.tensor_scalar_mul(attn_i, attn_i, recip[:])

                # stores go out on the SWDGE ring (Pool engine) so they don't
                # block later loads queued on the SP HWDGE ring; two halves so
                # the first half streams out while the second is computed
                attn_dst = attn_dram[b].rearrange("(i p) k -> p i k", p=P)
                half_t = QT // 2
                for piece in range(2):
                    s = slice(piece * half_t, (piece + 1) * half_t)
                    nc.gpsimd.dma_start(attn_dst[:, s, :], attn_sb[:, s, :])

            nc.gpsimd.dma_start(stats_dram[:], stats_sb[:])

    nc.compile()
    return nc


_NC_CACHE: dict = {}


def _get_nc():
    key = (B_LOC, MM_DT)
    if key not in _NC_CACHE:
        _NC_CACHE[key] = build_kernel()
    return _NC_CACHE[key]


def _np_in_dt():
    return np.dtype(mybir.dt.np(MM_DT))


def kernel(queries, keys, temperature, bias):
    in_dt = _np_in_dt()
    queries = np.ascontiguousarray(np.asarray(queries, dtype=np.float32))
    keys = np.ascontiguousarray(np.asarray(keys, dtype=np.float32))
    temperature = np.float32(np.asarray(temperature))
    bias = np.float32(np.asarray(bias))

    maskbias = np.where(keys[:, :, 0] == 0.0, np.float32(MASK_NEG), np.float32(0.0))

    q_in = queries.astype(in_dt, copy=False)
    k_in = keys.astype(in_dt, copy=False)
    mb_in = maskbias.astype(in_dt, copy=False)

    nc = _get_nc()
    in_maps = []
    for core in range(N_CORES):
        sl = slice(core * B_LOC, (core + 1) * B_LOC)
        in_maps.append(
            {
                "queries": q_in[sl],
                "keys": k_in[sl],
                "maskbias": mb_in[sl],
                "identity": np.eye(P, dtype=in_dt),
                "onesrow": np.ones((1, P), dtype=in_dt),
            }
        )
    res = bass_utils.run_bass_kernel_spmd(nc, in_maps, core_ids=list(range(N_CORES)))

    attn = np.concatenate([r["attn"] for r in res.results], axis=0)

    # stats: [P, B_LOC*2*QT] per core -> (-max, sum) per (b, q)
    neg_max = np.empty((B, Q), dtype=np.float32)
    row_sum = np.empty((B, Q), dtype=np.float32)
    for core, r in enumerate(res.results):
        st = r["stats"].reshape(P, B_LOC, QT, 2)
        for bl in range(B_LOC):
            bg = core * B_LOC + bl
            for i in range(QT):
                neg_max[bg, i * P : (i + 1) * P] = st[:, bl, i, 0]
                row_sum[bg, i * P : (i + 1) * P] = st[:, bl, i, 1]

    lse = np.log(row_sum, dtype=np.float32) - neg_max
    x = (lse + bias) * temperature
    confidence = (1.0 / (1.0 + np.exp(-x))).astype(np.float32)
    return attn, confidence


# revision 27
# speedup vs baseline: 1.3556x; 1.3556x over previous
"""Trainium2 Bass kernel for ContentAttention (sparse_attention).

reference semantics:
    logits = einsum("bqd,bkd->bqk", queries, keys)            # B,Q,K
    key_mask = (keys[:, :, 0] == 0.0)[:, None, :]             # mask keys whose feat0 == 0
    logits = where(key_mask, -inf, logits)
    lse = logsumexp(logits, -1)                               # B,Q
    confidence = sigmoid((lse + bias) * temperature)
    attn = softmax(logits, -1)
    returns (attn, confidence)

Strategy: data-parallel over batch; B=32 -> 4 batches per core on 8 cores.
Per batch on-device:
  - DMA Q[b], K[b] (512x1024) into SBUF in natural [q|k, d] layout
  - PE-transpose both into [d, q|k] layout (PSUM), evacuate to SBUF
    (ScalarE for Q^T, VectorE for K^T)
  - 4 output tiles [128q, 512k], each accumulating 8 d-chunk matmuls in PSUM
    + one rank-1 "mask bias" matmul (lhsT=ones[1,128], rhs=maskbias[1,512])
    which adds -1e30 to every masked key column (no-op when mask is empty)
  - softmax: DVE reduce_max(negate) -> ACT exp(x + negmax) with accum_out
    row-sum -> DVE reciprocal + tensor_scalar multiply
  - row (-max) and row sum are written to a stats tile; host finishes
    logsumexp + sigmoid for the confidence output (16K elements).

The whole Q/K path (DRAM tensors, SBUF tiles, identity, mask) is declared in
the matmul dtype: float32r (tf32-like, 1 PE cycle/row vs 4 for float32) by
default. The BIR verifier requires fp32r-matmul operands to be *produced* as
fp32r, so the dtype is set at allocation rather than bitcast at use.
"""

import os
import sys

if "/opt/trn_rl_repo" not in sys.path:
    sys.path.insert(0, "/opt/trn_rl_repo")

import numpy as np

import concourse.bass as bass
import concourse.mybir as mybir
import concourse.tile as tile
from concourse import bacc, bass_utils

B, Q, K, D = 32, 512, 512, 1024
N_CORES = 8
B_LOC = B // N_CORES  # batches per core
P = 128  # partitions
QT = Q // P  # q tiles per batch
DC = D // P  # d chunks

# matmul input dtype: float32r runs the PE at 1 cycle/row (vs 4 for float32)
# at ~tf32 precision. Overridable for experiments.
MM_DT_NAME = os.environ.get("ATTN_MM_DT", "f32r")
MM_DT = {
    "f32": mybir.dt.float32,
    "f32r": mybir.dt.float32r,
    "bf16": mybir.dt.bfloat16,
    "f16": mybir.dt.float16,
}[MM_DT_NAME]

# large negative logit for masked keys; must be representable in the matmul
# input dtype (fp16 tops out at 65504). exp(MASK_NEG - rowmax) == 0 either way
MASK_NEG = -60000.0 if MM_DT_NAME in ("f16", "bf16") else -1.0e30


def build_kernel(b_loc: int = B_LOC, mm_dt=MM_DT):
    nc = bacc.Bacc("TRN2", target_bir_lowering=False, debug=False)
    f32 = mybir.dt.float32

    q_dram = nc.dram_tensor("queries", (b_loc, Q, D), mm_dt, kind="ExternalInput")
    k_dram = nc.dram_tensor("keys", (b_loc, K, D), mm_dt, kind="ExternalInput")
    mb_dram = nc.dram_tensor("maskbias", (b_loc, K), mm_dt, kind="ExternalInput")
    # identity (for PE transpose) and a ones row (for the rank-1 mask matmul)
    # come in via DRAM: memset/affine_select can't produce f32r-typed tiles
    id_dram = nc.dram_tensor("identity", (P, P), mm_dt, kind="ExternalInput")
    ones_dram = nc.dram_tensor("onesrow", (1, P), mm_dt, kind="ExternalInput")
    attn_dram = nc.dram_tensor("attn", (b_loc, Q, K), f32, kind="ExternalOutput")
    # stats[p, b*2*QT + 2*i + 0] = -rowmax(b, q=128*i+p)
    # stats[p, b*2*QT + 2*i + 1] = rowsum(b, q=128*i+p)
    stats_dram = nc.dram_tensor("stats", (P, b_loc * 2 * QT), f32, kind="ExternalOutput")

    with tile.TileContext(nc) as tc:
        with (
            tc.tile_pool(name="consts", bufs=1) as consts,
            tc.tile_pool(name="nat", bufs=3) as nat_pool,
            tc.tile_pool(name="tsb", bufs=DC) as tsb_pool,
            tc.tile_pool(name="attn", bufs=2) as attn_pool,
            tc.tile_pool(name="small", bufs=4) as small_pool,
            tc.tile_pool(name="stats", bufs=1) as stats_pool,
            # a tps tile is [P, 2*Q]: one PSUM bank as fp16, two as f32/f32r
            tc.tile_pool(
                name="tps", bufs=(4 if mybir.dt.size(mm_dt) == 2 else 2), space="PSUM"
            ) as tps_pool,
            tc.tile_pool(name="lg", bufs=4, space="PSUM") as lg_pool,
        ):
            identity = consts.tile([P, P], mm_dt)
            nc.sync.dma_start(identity[:], id_dram[:])
            ones_row = consts.tile([1, P], mm_dt)
            nc.sync.dma_start(ones_row[:], ones_dram[:])
            mb_sb = consts.tile([1, b_loc * K], mm_dt)
            nc.sync.dma_start(
                mb_sb[:], mb_dram[:].rearrange("b k -> (b k)").unsqueeze(0)
            )
            stats_sb = stats_pool.tile([P, b_loc * 2 * QT], f32)

            for b in range(b_loc):
                # ---- load Q[b], K[b] in natural layout [p, i, d] (q = i*128+p)
                # batch 0 loads in d-slices: the pair-p transposes only need
                # d in [256p, 256p+256), so compute starts after the first
                # quarter of the first load instead of after all of it
                n_pieces = DC // 2 if b == 0 else 1
                q_nat = nat_pool.tile([P, QT, D], mm_dt, tag="qnat")
                k_nat = nat_pool.tile([P, QT, D], mm_dt, tag="knat")
                for nat, dram in ((q_nat, q_dram), (k_nat, k_dram)):
                    src = dram[b].rearrange("(i p) d -> p i d", p=P)
                    step = D // n_pieces
                    for piece in range(n_pieces):
                        s = slice(piece * step, (piece + 1) * step)
                        nc.sync.dma_start(nat[:, :, s], src[:, :, s])

                # ---- transpose to [d, q] / [d, k]; two d-chunks per psum tile
                # pair-major (Q0 K0 Q1 K1 ...) so the first matmuls' operands
                # are evacuated to SBUF while later pairs still transpose
                qt_sb = []
                kt_sb = []
                two_byte = mybir.dt.size(mm_dt) == 2
                for pair in range(DC // 2):
                    for nat, out_list, tag, on_dve in (
                        (q_nat, qt_sb, "qt", two_byte),
                        (k_nat, kt_sb, "kt", True),
                    ):
                        tps = tps_pool.tile([P, 2 * Q], mm_dt, tag="tps")
                        for half in range(2):
                            c = 2 * pair + half
                            for i in range(QT):
                                nc.tensor.transpose(
                                    tps[:, Q * half + P * i : Q * half + P * (i + 1)],
                                    nat[:, i, P * c : P * (c + 1)],
                                    identity[:],
                                )
                        t_sb = tsb_pool.tile([P, 2 * Q], mm_dt, tag=tag)
                        # 16-bit PSUM reads hit DVE 2x mode (0.66us vs 1.15us
                        # on ACT for [128,2048]); 4-byte dtypes split between
                        # the engines to balance load
                        if on_dve:
                            nc.vector.tensor_copy(t_sb[:], tps[:])
                        else:
                            nc.scalar.copy(t_sb[:], tps[:])
                        out_list.append(t_sb)

                # ---- matmuls + softmax per q tile; attn staged per q-tile so
                # each tile streams out as soon as it is normalized
                for i in range(QT):
                    lg = lg_pool.tile([P, K], f32, tag="lg")
                    for c in range(DC):
                        pair, half = divmod(c, 2)
                        nc.tensor.matmul(
                            lg[:],
                            qt_sb[pair][:, Q * half + P * i : Q * half + P * (i + 1)],
                            kt_sb[pair][:, Q * half : Q * (half + 1)],
                            start=(c == 0),
                            stop=False,
                        )
                    # mask bias: logits[q, k] += 1 * maskbias[k]
                    nc.tensor.matmul(
                        lg[:],
                        ones_row[0:1, :],
                        mb_sb[0:1, K * b : K * (b + 1)],
                        start=False,
                        stop=True,
                    )

                    col = b * 2 * QT + 2 * i
                    negmax = stats_sb[:, col : col + 1]
                    rowsum = stats_sb[:, col + 1 : col + 2]
                    nc.vector.reduce_max(
                        negmax, lg[:], axis=mybir.AxisListType.X, negate=True
                    )
                    attn_t = attn_pool.tile(
                        [P, K], f32, tag=f"attn{i}", name=f"attn{i}_{b}"
                    )
                    attn_i = attn_t[:]
                    nc.scalar.activation(
                        attn_i,
                        lg[:],
                        mybir.ActivationFunctionType.Exp,
                        bias=negmax,
                        scale=1.0,
                        accum_out=rowsum,
                    )
                    recip = small_pool.tile([P, 1], f32, tag="recip")
                    nc.vector.reciprocal(recip[:], rowsum)
                    nc.vector.tensor_scalar_mul(attn_i, attn_i, recip[:])

                    # store right away on the SWDGE ring (Pool engine) so
                    # stores don't block later loads on the SP HWDGE ring;
                    # the last batch has no loads behind it, so its stores use
                    # the (idle) SP HWDGE ring whose completion is faster
                    attn_dst = attn_dram[b].rearrange("(i p) k -> p i k", p=P)
                    store_eng = nc.sync if b == b_loc - 1 else nc.gpsimd
                    store_eng.dma_start(attn_dst[:, i, :], attn_i)

            nc.gpsimd.dma_start(stats_dram[:], stats_sb[:])

    nc.compile()
    return nc


_NC_CACHE: dict = {}


def _get_nc():
    key = (B_LOC, MM_DT)
    if key not in _NC_CACHE:
        _NC_CACHE[key] = build_kernel()
    return _NC_CACHE[key]


def _np_in_dt():
    return np.dtype(mybir.dt.np(MM_DT))


def kernel(queries, keys, temperature, bias):
    in_dt = _np_in_dt()
    queries = np.ascontiguousarray(np.asarray(queries, dtype=np.float32))
    keys = np.ascontiguousarray(np.asarray(keys, dtype=np.float32))
    temperature = np.float32(np.asarray(temperature))
    bias = np.float32(np.asarray(bias))

    maskbias = np.where(keys[:, :, 0] == 0.0, np.float32(MASK_NEG), np.float32(0.0))

    q_in = queries.astype(in_dt, copy=False)
    k_in = keys.astype(in_dt, copy=False)
    mb_in = maskbias.astype(in_dt, copy=False)

    nc = _get_nc()
    in_maps = []
    for core in range(N_CORES):
        sl = slice(core * B_LOC, (core + 1) * B_LOC)
        in_maps.append(
            {
                "queries": q_in[sl],
                "keys": k_in[sl],
                "maskbias": mb_in[sl],
                "identity": np.eye(P, dtype=in_dt),
                "onesrow": np.ones((1, P), dtype=in_dt),
            }
        )
    res = bass_utils.run_bass_kernel_spmd(nc, in_maps, core_ids=list(range(N_CORES)))

    attn = np.concatenate([r["attn"] for r in res.results], axis=0)

    # stats: [P, B_LOC*2*QT] per core -> (-max, sum) per (b, q)
    neg_max = np.empty((B, Q), dtype=np.float32)
    row_sum = np.empty((B, Q), dtype=np.float32)
    for core, r in enumerate(res.results):
        st = r["stats"].reshape(P, B_LOC, QT, 2)
        for bl in range(B_LOC):
            bg = core * B_LOC + bl
            for i in range(QT):
                neg_max[bg, i * P : (i + 1) * P] = st[:, bl, i, 0]
                row_sum[bg, i * P : (i + 1) * P] = st[:, bl, i, 1]

    lse = np.log(row_sum, dtype=np.float32) - neg_max
    x = (lse + bias) * temperature
    confidence = (1.0 / (1.0 + np.exp(-x))).astype(np.float32)
    return attn, confidence


# revision 35
# speedup vs baseline: 1.5299x; 1.1286x over previous
"""Trainium2 Bass kernel for ContentAttention (sparse_attention).

reference semantics:
    logits = einsum("bqd,bkd->bqk", queries, keys)            # B,Q,K
    key_mask = (keys[:, :, 0] == 0.0)[:, None, :]             # mask keys whose feat0 == 0
    logits = where(key_mask, -inf, logits)
    lse = logsumexp(logits, -1)                               # B,Q
    confidence = sigmoid((lse + bias) * temperature)
    attn = softmax(logits, -1)
    returns (attn, confidence)

Strategy: data-parallel over batch; B=32 -> 4 batches per core on 8 cores.
Per batch on-device:
  - DMA Q[b], K[b] (512x1024) into SBUF in natural [q|k, d] layout
  - PE-transpose both into [d, q|k] layout (PSUM), evacuate to SBUF
    (ScalarE for Q^T, VectorE for K^T)
  - 4 output tiles [128q, 512k], each accumulating 8 d-chunk matmuls in PSUM;
    when the input actually has masked keys, a rank-1 "mask bias" matmul
    (lhsT=ones[1,128], rhs=maskbias[1,512]) adds MASK_NEG to every masked key
    column (the unmasked variant skips it; host picks per input)
  - softmax: DVE reduce_max(negate) -> ACT exp(x + negmax) with accum_out
    row-sum -> DVE reciprocal + tensor_scalar multiply
  - row (-max) and row sum are written to a stats tile; host finishes
    logsumexp + sigmoid for the confidence output (16K elements).

The whole Q/K path (DRAM tensors, SBUF tiles, identity, mask) is declared in
the matmul dtype — float16 by default (1 PE cycle/row vs 4 for float32, and
half the input DMA). For the float32r variant the BIR verifier requires
fp32r-matmul operands to be *produced* as fp32r, so the dtype is set at
allocation rather than bitcast at use.
"""

import os
import sys

if "/opt/trn_rl_repo" not in sys.path:
    sys.path.insert(0, "/opt/trn_rl_repo")

import numpy as np

import concourse.bass as bass
import concourse.mybir as mybir
import concourse.tile as tile
from concourse import bacc, bass_utils

B, Q, K, D = 32, 512, 512, 1024
N_CORES = 8
B_LOC = B // N_CORES  # batches per core
P = 128  # partitions
QT = Q // P  # q tiles per batch
DC = D // P  # d chunks

# matmul input dtype. float16 (default): PE runs at 1 cycle/row (vs 4 for
# float32) and input DMA halves; measured attn L2 rel err 1.5e-3 on the
# reference inputs. float32r: same PE rate, fp32-sized DMA, rel err 7.6e-4.
MM_DT_NAME = os.environ.get("ATTN_MM_DT", "f16")
MM_DT = {
    "f32": mybir.dt.float32,
    "f32r": mybir.dt.float32r,
    "bf16": mybir.dt.bfloat16,
    "f16": mybir.dt.float16,
}[MM_DT_NAME]

# large negative logit for masked keys; must be representable in the matmul
# input dtype (fp16 tops out at 65504). exp(MASK_NEG - rowmax) == 0 either way
MASK_NEG = -60000.0 if MM_DT_NAME in ("f16", "bf16") else -1.0e30


def build_kernel(b_loc: int = B_LOC, mm_dt=MM_DT, masked: bool = True):
    nc = bacc.Bacc("TRN2", target_bir_lowering=False, debug=False)
    f32 = mybir.dt.float32

    q_dram = nc.dram_tensor("queries", (b_loc, Q, D), mm_dt, kind="ExternalInput")
    k_dram = nc.dram_tensor("keys", (b_loc, K, D), mm_dt, kind="ExternalInput")
    # identity (for PE transpose) and a ones row (for the rank-1 mask matmul)
    # come in via DRAM: memset/affine_select can't produce f32r-typed tiles
    id_dram = nc.dram_tensor("identity", (P, P), mm_dt, kind="ExternalInput")
    if masked:
        mb_dram = nc.dram_tensor("maskbias", (b_loc, K), mm_dt, kind="ExternalInput")
        ones_dram = nc.dram_tensor("onesrow", (1, P), mm_dt, kind="ExternalInput")
    attn_dram = nc.dram_tensor("attn", (b_loc, Q, K), f32, kind="ExternalOutput")
    # stats[p, b*2*QT + 2*i + 0] = -rowmax(b, q=128*i+p)
    # stats[p, b*2*QT + 2*i + 1] = rowsum(b, q=128*i+p)
    stats_dram = nc.dram_tensor("stats", (P, b_loc * 2 * QT), f32, kind="ExternalOutput")

    with tile.TileContext(nc) as tc:
        with (
            tc.tile_pool(name="consts", bufs=1) as consts,
            tc.tile_pool(name="nat", bufs=3) as nat_pool,
            tc.tile_pool(name="tsb", bufs=DC) as tsb_pool,
            tc.tile_pool(name="attn", bufs=2) as attn_pool,
            tc.tile_pool(name="small", bufs=4) as small_pool,
            tc.tile_pool(name="stats", bufs=1) as stats_pool,
            # a tps tile is [P, 2*Q]: one PSUM bank as fp16, two as f32/f32r
            tc.tile_pool(
                name="tps", bufs=(4 if mybir.dt.size(mm_dt) == 2 else 2), space="PSUM"
            ) as tps_pool,
            tc.tile_pool(name="lg", bufs=4, space="PSUM") as lg_pool,
        ):
            # const loads ride the SWDGE ring so the first Q/K pieces are
            # not queued behind them on SP
            identity = consts.tile([P, P], mm_dt)
            nc.gpsimd.dma_start(identity[:], id_dram[:])
            if masked:
                ones_row = consts.tile([1, P], mm_dt)
                nc.gpsimd.dma_start(ones_row[:], ones_dram[:])
                mb_sb = consts.tile([1, b_loc * K], mm_dt)
                nc.gpsimd.dma_start(
                    mb_sb[:], mb_dram[:].rearrange("b k -> (b k)").unsqueeze(0)
                )
            stats_sb = stats_pool.tile([P, b_loc * 2 * QT], f32)

            for b in range(b_loc):
                # ---- load Q[b], K[b] in natural layout [p, i, d] (q = i*128+p)
                # batch 0 loads in d-slices: the pair-p transposes only need
                # d in [256p, 256p+256), so compute starts after the first
                # quarter of the first load instead of after all of it
                n_pieces = DC // 2 if b == 0 else 1
                q_nat = nat_pool.tile([P, QT, D], mm_dt, tag="qnat")
                k_nat = nat_pool.tile([P, QT, D], mm_dt, tag="knat")
                step = D // n_pieces
                for piece in range(n_pieces):
                    s = slice(piece * step, (piece + 1) * step)
                    for nat, dram in ((q_nat, q_dram), (k_nat, k_dram)):
                        src = dram[b].rearrange("(i p) d -> p i d", p=P)
                        nc.sync.dma_start(nat[:, :, s], src[:, :, s])

                # ---- transpose to [d, q] / [d, k]; two d-chunks per psum tile
                # pair-major (Q0 K0 Q1 K1 ...) so the first matmuls' operands
                # are evacuated to SBUF while later pairs still transpose
                qt_sb = []
                kt_sb = []
                two_byte = mybir.dt.size(mm_dt) == 2
                for pair in range(DC // 2):
                    for nat, out_list, tag, on_dve in (
                        (q_nat, qt_sb, "qt", two_byte),
                        (k_nat, kt_sb, "kt", True),
                    ):
                        tps = tps_pool.tile([P, 2 * Q], mm_dt, tag="tps")
                        for half in range(2):
                            c = 2 * pair + half
                            for i in range(QT):
                                nc.tensor.transpose(
                                    tps[:, Q * half + P * i : Q * half + P * (i + 1)],
                                    nat[:, i, P * c : P * (c + 1)],
                                    identity[:],
                                )
                        t_sb = tsb_pool.tile([P, 2 * Q], mm_dt, tag=tag)
                        # 16-bit PSUM reads hit DVE 2x mode (0.66us vs 1.15us
                        # on ACT for [128,2048]); 4-byte dtypes split between
                        # the engines to balance load
                        if on_dve:
                            nc.vector.tensor_copy(t_sb[:], tps[:])
                        else:
                            nc.scalar.copy(t_sb[:], tps[:])
                        out_list.append(t_sb)

                # ---- matmuls + softmax per q tile; attn staged per q-tile so
                # each tile streams out as soon as it is normalized
                for i in range(QT):
                    lg = lg_pool.tile([P, K], f32, tag="lg")
                    for c in range(DC):
                        pair, half = divmod(c, 2)
                        nc.tensor.matmul(
                            lg[:],
                            qt_sb[pair][:, Q * half + P * i : Q * half + P * (i + 1)],
                            kt_sb[pair][:, Q * half : Q * (half + 1)],
                            start=(c == 0),
                            stop=(not masked and c == DC - 1),
                        )
                    if masked:
                        # mask bias: logits[q, k] += 1 * maskbias[k]
                        nc.tensor.matmul(
                            lg[:],
                            ones_row[0:1, :],
                            mb_sb[0:1, K * b : K * (b + 1)],
                            start=False,
                            stop=True,
                        )

                    col = b * 2 * QT + 2 * i
                    negmax = stats_sb[:, col : col + 1]
                    rowsum = stats_sb[:, col + 1 : col + 2]
                    nc.vector.reduce_max(
                        negmax, lg[:], axis=mybir.AxisListType.X, negate=True
                    )
                    attn_t = attn_pool.tile(
                        [P, K], f32, tag=f"attn{i}", name=f"attn{i}_{b}"
                    )
                    attn_i = attn_t[:]
                    nc.scalar.activation(
                        attn_i,
                        lg[:],
                        mybir.ActivationFunctionType.Exp,
                        bias=negmax,
                        scale=1.0,
                        accum_out=rowsum,
                    )
                    recip = small_pool.tile([P, 1], f32, tag="recip")
                    nc.vector.reciprocal(recip[:], rowsum)
                    nc.vector.tensor_scalar_mul(attn_i, attn_i, recip[:])

                    # store right away on the SWDGE ring (Pool engine) so
                    # stores don't block later loads on the SP HWDGE ring;
                    # the last batch has no loads behind it, so its stores use
                    # the (idle) SP HWDGE ring whose completion is faster
                    attn_dst = attn_dram[b].rearrange("(i p) k -> p i k", p=P)
                    store_eng = nc.sync if b == b_loc - 1 else nc.gpsimd
                    store_eng.dma_start(attn_dst[:, i, :], attn_i)

            nc.gpsimd.dma_start(stats_dram[:], stats_sb[:])

    nc.compile()
    return nc


_NC_CACHE: dict = {}


def _get_nc(masked: bool):
    key = (B_LOC, MM_DT, masked)
    if key not in _NC_CACHE:
        _NC_CACHE[key] = build_kernel(masked=masked)
    return _NC_CACHE[key]


def _np_in_dt():
    return np.dtype(mybir.dt.np(MM_DT))


def kernel(queries, keys, temperature, bias):
    in_dt = _np_in_dt()
    queries = np.ascontiguousarray(np.asarray(queries, dtype=np.float32))
    keys = np.ascontiguousarray(np.asarray(keys, dtype=np.float32))
    temperature = np.float32(np.asarray(temperature))
    bias = np.float32(np.asarray(bias))

    key_mask = keys[:, :, 0] == 0.0
    masked = bool(key_mask.any())

    q_in = queries.astype(in_dt, copy=False)
    k_in = keys.astype(in_dt, copy=False)

    nc = _get_nc(masked)
    in_maps = []
    for core in range(N_CORES):
        sl = slice(core * B_LOC, (core + 1) * B_LOC)
        im = {
            "queries": q_in[sl],
            "keys": k_in[sl],
            "identity": np.eye(P, dtype=in_dt),
        }
        if masked:
            mb = np.where(key_mask[sl], np.float32(MASK_NEG), np.float32(0.0))
            im["maskbias"] = mb.astype(in_dt, copy=False)
            im["onesrow"] = np.ones((1, P), dtype=in_dt)
        in_maps.append(im)
    res = bass_utils.run_bass_kernel_spmd(nc, in_maps, core_ids=list(range(N_CORES)))

    attn = np.concatenate([r["attn"] for r in res.results], axis=0)

    # stats: [P, B_LOC*2*QT] per core -> (-max, sum) per (b, q)
    neg_max = np.empty((B, Q), dtype=np.float32)
    row_sum = np.empty((B, Q), dtype=np.float32)
    for core, r in enumerate(res.results):
        st = r["stats"].reshape(P, B_LOC, QT, 2)
        for bl in range(B_LOC):
            bg = core * B_LOC + bl
            for i in range(QT):
                neg_max[bg, i * P : (i + 1) * P] = st[:, bl, i, 0]
                row_sum[bg, i * P : (i + 1) * P] = st[:, bl, i, 1]

    lse = np.log(row_sum, dtype=np.float32) - neg_max
    x = (lse + bias) * temperature
    confidence = (1.0 / (1.0 + np.exp(-x))).astype(np.float32)
    return attn, confidence


# revision 43
# speedup vs baseline: 1.9034x; 1.2441x over previous
"""Trainium2 Bass kernel for ContentAttention (sparse_attention).

reference semantics:
    logits = einsum("bqd,bkd->bqk", queries, keys)            # B,Q,K
    key_mask = (keys[:, :, 0] == 0.0)[:, None, :]             # mask keys whose feat0 == 0
    logits = where(key_mask, -inf, logits)
    lse = logsumexp(logits, -1)                               # B,Q
    confidence = sigmoid((lse + bias) * temperature)
    attn = softmax(logits, -1)
    returns (attn, confidence)

Strategy: data-parallel over batch; B=32 -> 4 batches per core on 8 cores.

The host marshals the inputs once per call: fp32 -> fp16 cast (the PE runs
fp16 matmuls at 1 cycle/row vs 4 for fp32; measured attn L2 rel err 1.5e-3)
fused with a (B, S, D) -> (B, D, S) transpose, so the device receives Q^T/K^T
with the contraction dim d already on partitions. That removes all on-device
PE transposes and their PSUM-evacuation copies; the device kernel is pure
matmul + fused softmax:

  per batch:
  - DMA Q^T[b], K^T[b] (1 MB fp16 each) into SBUF tiles [128 d_lo, 8 d_hi, 512]
  - 4 logits tiles [128q, 512k], each accumulating 8 d-chunk matmuls in PSUM;
    when the input actually has masked keys, a rank-1 "mask bias" matmul
    (lhsT=ones[1,128], rhs=maskbias[1,512]) adds MASK_NEG to every masked key
    column (the unmasked variant skips it; the host picks per input)
  - softmax: DVE reduce_max(negate) -> ACT exp(x + negmax) with accum_out
    row-sum -> DVE reciprocal + tensor_scalar multiply -> per-tile store
  - row (-max) and row sum go to a stats tile; the host finishes
    logsumexp + sigmoid for the confidence output (16K elements).
"""

import os
import sys

if "/opt/trn_rl_repo" not in sys.path:
    sys.path.insert(0, "/opt/trn_rl_repo")

import numpy as np

import concourse.bass as bass
import concourse.mybir as mybir
import concourse.tile as tile
from concourse import bacc, bass_utils

B, Q, K, D = 32, 512, 512, 1024
N_CORES = 8
B_LOC = B // N_CORES  # batches per core
P = 128  # partitions
QT = Q // P  # q tiles per batch
DC = D // P  # d chunks

# matmul input dtype. float16 (default): PE runs at 1 cycle/row (vs 4 for
# float32) and input DMA halves; measured attn L2 rel err 1.5e-3 on the
# reference inputs. float32r: same PE rate, fp32-sized DMA, rel err 7.6e-4.
MM_DT_NAME = os.environ.get("ATTN_MM_DT", "f16")
MM_DT = {
    "f32": mybir.dt.float32,
    "f32r": mybir.dt.float32r,
    "bf16": mybir.dt.bfloat16,
    "f16": mybir.dt.float16,
}[MM_DT_NAME]

# large negative logit for masked keys; must be representable in the matmul
# input dtype (fp16 tops out at 65504). exp(MASK_NEG - rowmax) == 0 either way
MASK_NEG = -60000.0 if MM_DT_NAME in ("f16", "bf16") else -1.0e30


def build_kernel(b_loc: int = B_LOC, mm_dt=MM_DT, masked: bool = True):
    nc = bacc.Bacc("TRN2", target_bir_lowering=False, debug=False)
    f32 = mybir.dt.float32

    # inputs arrive pre-transposed: queries_t[b, d, q], keys_t[b, d, k]
    qt_dram = nc.dram_tensor("queries_t", (b_loc, D, Q), mm_dt, kind="ExternalInput")
    kt_dram = nc.dram_tensor("keys_t", (b_loc, D, K), mm_dt, kind="ExternalInput")
    if masked:
        mb_dram = nc.dram_tensor("maskbias", (b_loc, K), mm_dt, kind="ExternalInput")
        ones_dram = nc.dram_tensor("onesrow", (1, P), mm_dt, kind="ExternalInput")
    # attn streams out as fp16 when the matmul dtype is 16-bit (halves store
    # traffic; adds <= 2^-12 abs error to in-[0,1] values, far below the
    # matmul rounding) and the host upcasts to fp32
    out_dt = mm_dt if mybir.dt.size(mm_dt) == 2 else f32
    attn_dram = nc.dram_tensor("attn", (b_loc, Q, K), out_dt, kind="ExternalOutput")
    # stats[p, b*2*QT + 2*i + 0] = -rowmax(b, q=128*i+p)
    # stats[p, b*2*QT + 2*i + 1] = rowsum(b, q=128*i+p)
    stats_dram = nc.dram_tensor("stats", (P, b_loc * 2 * QT), f32, kind="ExternalOutput")

    with tile.TileContext(nc) as tc:
        with (
            tc.tile_pool(name="consts", bufs=1) as consts,
            tc.tile_pool(name="tin", bufs=3) as tin_pool,
            tc.tile_pool(name="attn", bufs=2) as attn_pool,
            tc.tile_pool(name="small", bufs=4) as small_pool,
            tc.tile_pool(name="stats", bufs=1) as stats_pool,
            tc.tile_pool(name="lg", bufs=8, space="PSUM") as lg_pool,
        ):
            if masked:
                # const loads ride the SWDGE ring so the first Q/K pieces are
                # not queued behind them on SP
                ones_row = consts.tile([1, P], mm_dt)
                nc.gpsimd.dma_start(ones_row[:], ones_dram[:])
                mb_sb = consts.tile([1, b_loc * K], mm_dt)
                nc.gpsimd.dma_start(
                    mb_sb[:], mb_dram[:].rearrange("b k -> (b k)").unsqueeze(0)
                )
            stats_sb = stats_pool.tile([P, b_loc * 2 * QT], f32)

            for b in range(b_loc):
                # ---- load Q^T[b], K^T[b] as [p, c, s]: d = c*128 + p
                # batch 0 loads in d-chunk pieces so the first accumulating
                # matmuls start after the first quarter of the first load
                piece_chunks = [2, 2, 2, 2] if b == 0 else [DC]
                qt_sb = tin_pool.tile([P, DC, Q], mm_dt, tag="qt")
                kt_sb = tin_pool.tile([P, DC, K], mm_dt, tag="kt")
                c0 = 0
                for nch in piece_chunks:
                    s = slice(c0, c0 + nch)
                    c0 += nch
                    for t_sb, dram in ((qt_sb, qt_dram), (kt_sb, kt_dram)):
                        src = dram[b].rearrange("(c p) s -> p c s", p=P)
                        nc.sync.dma_start(t_sb[:, s, :], src[:, s, :])

                # ---- matmuls + softmax per q tile; attn staged per q-tile so
                # each tile streams out as soon as it is normalized
                for i in range(QT):
                    lg = lg_pool.tile([P, K], f32, tag="lg")
                    for c in range(DC):
                        nc.tensor.matmul(
                            lg[:],
                            qt_sb[:, c, P * i : P * (i + 1)],
                            kt_sb[:, c, :],
                            start=(c == 0),
                            stop=(not masked and c == DC - 1),
                        )
                    if masked:
                        # mask bias: logits[q, k] += 1 * maskbias[k]
                        nc.tensor.matmul(
                            lg[:],
                            ones_row[0:1, :],
                            mb_sb[0:1, K * b : K * (b + 1)],
                            start=False,
                            stop=True,
                        )

                    col = b * 2 * QT + 2 * i
                    negmax = stats_sb[:, col : col + 1]
                    rowsum = stats_sb[:, col + 1 : col + 2]
                    nc.vector.reduce_max(
                        negmax, lg[:], axis=mybir.AxisListType.X, negate=True
                    )
                    attn_t = attn_pool.tile(
                        [P, K], out_dt, tag=f"attn{i}", name=f"attn{i}_{b}"
                    )
                    attn_i = attn_t[:]
                    nc.scalar.activation(
                        attn_i,
                        lg[:],
                        mybir.ActivationFunctionType.Exp,
                        bias=negmax,
                        scale=1.0,
                        accum_out=rowsum,
                    )
                    recip = small_pool.tile([P, 1], f32, tag="recip")
                    nc.vector.reciprocal(recip[:], rowsum)
                    last_batch = b == b_loc - 1
                    nc.vector.tensor_scalar_mul(attn_i, attn_i, recip[:])

                    # store right away on the SWDGE ring (Pool engine) so
                    # stores don't block later loads on the SP HWDGE ring;
                    # the last batch has no loads behind it, so its stores use
                    # the (idle) SP HWDGE ring whose completion is faster
                    attn_dst = attn_dram[b].rearrange("(i p) k -> p i k", p=P)
                    store_eng = nc.sync if last_batch else nc.gpsimd
                    store_eng.dma_start(attn_dst[:, i, :], attn_i)

            nc.gpsimd.dma_start(stats_dram[:], stats_sb[:])

    nc.compile()
    return nc


_NC_CACHE: dict = {}


def _get_nc(masked: bool):
    key = (B_LOC, MM_DT, masked)
    if key not in _NC_CACHE:
        _NC_CACHE[key] = build_kernel(masked=masked)
    return _NC_CACHE[key]


def _np_in_dt():
    return np.dtype(mybir.dt.np(MM_DT))


def kernel(queries, keys, temperature, bias):
    in_dt = _np_in_dt()
    queries = np.asarray(queries)
    keys = np.asarray(keys)
    temperature = np.float32(np.asarray(temperature))
    bias = np.float32(np.asarray(bias))

    key_mask = np.asarray(keys[:, :, 0] == 0.0)
    masked = bool(key_mask.any())

    # host-side marshalling: cast to the matmul dtype fused with the
    # (B, S, D) -> (B, D, S) transpose the device layout wants
    q_in = np.ascontiguousarray(queries.transpose(0, 2, 1).astype(in_dt))
    k_in = np.ascontiguousarray(keys.transpose(0, 2, 1).astype(in_dt))

    nc = _get_nc(masked)
    in_maps = []
    for core in range(N_CORES):
        sl = slice(core * B_LOC, (core + 1) * B_LOC)
        im = {
            "queries_t": q_in[sl],
            "keys_t": k_in[sl],
        }
        if masked:
            mb = np.where(key_mask[sl], np.float32(MASK_NEG), np.float32(0.0))
            im["maskbias"] = mb.astype(in_dt, copy=False)
            im["onesrow"] = np.ones((1, P), dtype=in_dt)
        in_maps.append(im)
    res = bass_utils.run_bass_kernel_spmd(nc, in_maps, core_ids=list(range(N_CORES)))

    attn = np.concatenate([r["attn"] for r in res.results], axis=0)

    # stats: [P, B_LOC*2*QT] per core -> (-max, sum) per (b, q)
    neg_max = np.empty((B, Q), dtype=np.float32)
    row_sum = np.empty((B, Q), dtype=np.float32)
    for core, r in enumerate(res.results):
        st = r["stats"].reshape(P, B_LOC, QT, 2)
        for bl in range(B_LOC):
            bg = core * B_LOC + bl
            for i in range(QT):
                neg_max[bg, i * P : (i + 1) * P] = st[:, bl, i, 0]
                row_sum[bg, i * P : (i + 1) * P] = st[:, bl, i, 1]

    lse = np.log(row_sum, dtype=np.float32) - neg_max
    x = (lse + bias) * temperature
    confidence = (1.0 / (1.0 + np.exp(-x))).astype(np.float32)
    return attn, confidence
